# revision 14
# baseline (speedup 1.0000x reference)
"""CRF layer gradient kernel for 8 TRN2 NeuronCores (v3).

Data-parallel over N=2048 words (256/core = 4 chains x 64 words).
Scaled forward-backward in exp domain, ETs = exp(T)/c:
  AE[i+1] = (AE[i] @ ETs) * E[i+1]   (AE[0] = E[0])
  BE[i-1] = (ETs @ BE[i]) * E[i-1]   (BE[63] = E[63], unnormalized)
fw/bw scans run CONCURRENTLY (z = sum_k AE[63] folded in afterwards via
ez = einv * (1/z) broadcast). Packing [128 = 4c x 32k, 4096 = 64 pos x 64 wg]
(POSITION-major: scan slices contiguous).

v3 perf structure:
- dt loaded as 8 x 2MB q-major units (host layout [8,128,8192]) split
  across both HWDGE rings (sync: q0-3, scalar: q7-4) -> big DMAs at line rate.
- dn loaded as 16 x 1MB partition-major units on the sync ring after dt.
- scan EMITTED INTERLEAVED with dots units so the PE FIFO never blocks the
  scan behind not-yet-needed dots matmuls; scan trails dt arrival.
- scan state in bf16 (fast matmuls), dots/dw matmuls col-tiled bf16.
- phase E per 128-column block: p1 -> G -> PE transpose -> col-tiled dw
  accumulation; dw reduced on device; ae/be out in bf16 on gpsimd ring.
"""

import sys

import numpy as np

sys.path.insert(0, "/opt/trn_rl_repo")

import concourse.bass as bass
import concourse.tile as tile
from concourse import bacc, mybir
from concourse.bass_utils import run_bass_kernel_spmd

N, M, K, D = 2048, 64, 32, 512
NC = 8
WPC = N // NC          # 256 words per core
RPC = WPC * M          # 16384 rows per core
CHAT = 60.0
F32 = mybir.dt.float32
BF16 = mybir.dt.bfloat16

_CACHE = {}


def _build_module():
    nc = bacc.Bacc("TRN2", target_bir_lowering=False, debug=False)

    dt_d = nc.dram_tensor("dt", [8, 128, 8192], BF16, kind="ExternalInput")
    dn_d = nc.dram_tensor("dn", [128, 65536], BF16, kind="ExternalInput")
    wt4_d = nc.dram_tensor("wt4", [128, 4, 128], BF16, kind="ExternalInput")
    etf_d = nc.dram_tensor("etf", [128, 128], BF16, kind="ExternalInput")
    etb_d = nc.dram_tensor("etb", [128, 128], BF16, kind="ExternalInput")
    oz_d = nc.dram_tensor("oz", [128, 4], BF16, kind="ExternalInput")
    ob_d = nc.dram_tensor("ob", [4, 128], F32, kind="ExternalInput")
    on32_d = nc.dram_tensor("on32", [128, K], F32, kind="ExternalInput")
    id128_d = nc.dram_tensor("id128", [128, 128], BF16, kind="ExternalInput")
    oh_d = nc.dram_tensor("oh", [128, 4096], BF16, kind="ExternalInput")
    dw_d = nc.dram_tensor("dw", [K, D], F32, kind="ExternalOutput")
    ae_d = nc.dram_tensor("ae", [128, 4096], BF16, kind="ExternalOutput")
    be_d = nc.dram_tensor("be", [128, 4096], BF16, kind="ExternalOutput")

    with tile.TileContext(nc) as tc:
        _kernel_body(tc, nc, dt_d, dn_d, wt4_d, etf_d, etb_d, oz_d, ob_d,
                     on32_d, id128_d, oh_d, dw_d, ae_d, be_d)
    nc.compile()
    return nc


def _kernel_body(tc, nc, dt_d, dn_d, wt4_d, etf_d, etb_d, oz_d, ob_d,
                 on32_d, id128_d, oh_d, dw_d, ae_d, be_d):
    from contextlib import ExitStack
    ctx = ExitStack()
    with ctx:
        consts = ctx.enter_context(tc.tile_pool(name="consts", bufs=1))
        big = ctx.enter_context(tc.tile_pool(name="big", bufs=1))
        dtp = ctx.enter_context(tc.tile_pool(name="dtp", bufs=5))
        dnp = ctx.enter_context(tc.tile_pool(name="dnp", bufs=8))
        scr = ctx.enter_context(tc.tile_pool(name="scr", bufs=6))
        gsbp = ctx.enter_context(tc.tile_pool(name="gsbp", bufs=3))

        # small consts first on the sync HWDGE ring (fast, ~1us total)
        wt4_t = consts.tile([128, 4, 128], BF16)
        nc.sync.dma_start(wt4_t[:], wt4_d.ap())
        etf_t = consts.tile([128, 128], BF16)
        nc.sync.dma_start(etf_t[:], etf_d.ap())
        etb_t = consts.tile([128, 128], BF16)
        nc.sync.dma_start(etb_t[:], etb_d.ap())
        oz_t = consts.tile([128, 4], BF16)
        nc.sync.dma_start(oz_t[:], oz_d.ap())
        ob_t = consts.tile([4, 128], F32)
        nc.sync.dma_start(ob_t[:], ob_d.ap())
        on32_t = consts.tile([128, K], F32)
        nc.sync.dma_start(on32_t[:], on32_d.ap())
        id128_t = consts.tile([128, 128], BF16)
        nc.sync.dma_start(id128_t[:], id128_d.ap())
        oh_t = big.tile([128, 4096], BF16, tag="oh")

        e_t = big.tile([128, 4096], BF16, tag="e")
        ez_t = big.tile([128, 4096], BF16, tag="ez")   # einv, then einv*rz
        ae_t = big.tile([128, 4096], BF16, tag="ae")
        be_t = big.tile([128, 4096], BF16, tag="be")
        rzb_t = consts.tile([128, 64], F32)
        rz_t = consts.tile([4, 64], F32)

        # ---- input DMAs: dt on both rings, dn on sync after dt ----
        dt_tiles = {}
        for q in (0, 1, 2, 3):
            dt_tiles[q] = dtp.tile([128, 8192], BF16, tag="dt", name=f"dt{q}")
            nc.sync.dma_start(dt_tiles[q][:], dt_d.ap()[q])
        for q in (7, 6, 5, 4):
            dt_tiles[q] = dtp.tile([128, 8192], BF16, tag="dt", name=f"dt{q}")
            nc.scalar.dma_start(dt_tiles[q][:], dt_d.ap()[q])
        # oh only needed in phase E; load it on the scalar ring after dt
        nc.scalar.dma_start(oh_t[:], oh_d.ap())
        dn_tiles = {}
        for u in (0, 4, 8, 12, 1, 5, 9, 13, 2, 6, 10, 14, 3, 7, 11, 15):
            dn_tiles[u] = dnp.tile([128, 8, 512], BF16, tag="dn",
                                   name=f"dn{u}")
            nc.sync.dma_start(
                dn_tiles[u][:],
                dn_d.ap()[:, 4096 * u:4096 * u + 4096]
                .rearrange("p (j d) -> p j d", j=8))

        dot_psum = {}

        def emit_unit(dotp, q):
            """dots for positions 8q..8q+8: 16 col-tiled MMs + exp ACTs."""
            P = dotp.tile([128, 512], F32)
            dot_psum[q] = P
            for g in range(4):
                for c in range(4):
                    nc.tensor.matmul(
                        P[32 * c:32 * c + 32, :],
                        wt4_t[:, g, 32 * c:32 * c + 32],
                        dt_tiles[q][:, 512 * (4 * c + g):512 * (4 * c + g) + 512],
                        start=(g == 0), stop=(g == 3),
                        tile_position=(0, 32 * c))
            sl = slice(512 * q, 512 * q + 512)
            nc.scalar.activation(e_t[:, sl], P[:],
                                 mybir.ActivationFunctionType.Exp)
            nc.scalar.activation(ez_t[:, sl], P[:],
                                 mybir.ActivationFunctionType.Exp,
                                 scale=-1.0)

        # ---- Phase A+B interleaved: dots units woven into the scan ----
        with tc.tile_pool(name="dotp", bufs=3, space="PSUM") as dotp, \
             tc.tile_pool(name="scanp", bufs=3, space="PSUM") as scanp, \
             tc.tile_pool(name="zp", bufs=1, space="PSUM") as zp:
            emit_unit(dotp, 0)
            emit_unit(dotp, 7)

            nc.vector.tensor_copy(ae_t[:, 0:64], e_t[:, 0:64])
            nc.vector.tensor_copy(be_t[:, 4032:4096], e_t[:, 4032:4096])
            af = scanp.tile([128, 64], F32, tag="s")
            nc.tensor.matmul(af[:], etf_t[:], ae_t[:, 0:64],
                             start=True, stop=True)
            bb = scanp.tile([128, 64], F32, tag="s")
            nc.tensor.matmul(bb[:], etb_t[:], be_t[:, 4032:4096],
                             start=True, stop=True)

            unit_sched = {8: (1, 6), 16: (2, 5), 24: (3, 4)}
            for s in range(1, 64):
                if s in unit_sched:
                    for q in unit_sched[s]:
                        emit_unit(dotp, q)
                sf = slice(64 * s, 64 * s + 64)
                sb = slice(64 * (63 - s), 64 * (63 - s) + 64)
                nc.vector.tensor_mul(ae_t[:, sf], af[:], e_t[:, sf])
                nc.vector.tensor_mul(be_t[:, sb], bb[:], e_t[:, sb])
                if s < 63:
                    af = scanp.tile([128, 64], F32, tag="s")
                    nc.tensor.matmul(af[:], etf_t[:], ae_t[:, sf],
                                     start=True, stop=True)
                    bb = scanp.tile([128, 64], F32, tag="s")
                    nc.tensor.matmul(bb[:], etb_t[:], be_t[:, sb],
                                     start=True, stop=True)

            nc.scalar.dma_start(ae_d.ap(), ae_t[:])
            nc.scalar.dma_start(be_d.ap(), be_t[:])

            # ---- z and ez = einv * (1/z) broadcast ----
            z_ps = zp.tile([128, 64], F32, tag="z")
            nc.tensor.matmul(z_ps[0:4, :], oz_t[:], ae_t[:, 4032:4096],
                             start=True, stop=True)
            nc.vector.reciprocal(rz_t[:], z_ps[0:4, :])
            rzb_ps = zp.tile([128, 64], F32, tag="z")
            nc.tensor.matmul(rzb_ps[:], ob_t[:], rz_t[:],
                             start=True, stop=True)
            nc.vector.tensor_copy(rzb_t[:], rzb_ps[:])

        ez3 = ez_t[:].rearrange("p (i w) -> p i w", i=64)
        rz3 = rzb_t[:].unsqueeze(1)
        rz3b, ez3b = bass.broadcast_tensor_aps(rz3, ez3)
        nc.vector.tensor_mul(ez3, ez3b, rz3b)        # in-place einv -> ez

        # ---- Phase E: G = p1 - oh; PE transpose; col-tiled dw ----
        with tc.tile_pool(name="trp", bufs=2, space="PSUM") as trp, \
             tc.tile_pool(name="dwp", bufs=1, space="PSUM") as dwp, \
             tc.tile_pool(name="drp", bufs=1, space="PSUM") as drp:
            dwacc = dwp.tile([128, 512], F32)
            for jj in range(32):
                sl = slice(128 * jj, 128 * jj + 128)
                p1c = scr.tile([128, 128], F32, tag="p1")
                nc.vector.tensor_mul(p1c[:], ae_t[:, sl], be_t[:, sl])
                p1b = scr.tile([128, 128], BF16, tag="p1b")
                nc.vector.tensor_mul(p1b[:], p1c[:], ez_t[:, sl])
                gc = scr.tile([128, 128], BF16, tag="g")
                nc.vector.tensor_sub(gc[:], p1b[:], oh_t[:, sl])
                tr = trp.tile([128, 128], BF16)
                nc.tensor.transpose(tr[:], gc[:], id128_t[:])
                gsb = gsbp.tile([128, 128], BF16)
                nc.scalar.activation(gsb[:], tr[:],
                                     mybir.ActivationFunctionType.Copy)
                for c in range(4):
                    j = 32 * c + jj
                    u, slot = j // 8, j % 8
                    nc.tensor.matmul(dwacc[32 * c:32 * c + 32, :],
                                     gsb[:, 32 * c:32 * c + 32],
                                     dn_tiles[u][:, slot, :],
                                     start=(jj == 0), stop=(jj == 31),
                                     tile_position=(0, 32 * c))

            dwsb = gsbp.tile([128, 512], F32, tag="dwsb")
            nc.vector.tensor_copy(dwsb[:], dwacc[:])
            dwred = drp.tile([K, 512], F32)
            nc.tensor.matmul(dwred[:], on32_t[:], dwsb[:],
                             start=True, stop=True)
            dwout = gsbp.tile([K, 512], F32, tag="dwout")
            nc.vector.tensor_copy(dwout[:], dwred[:])
            nc.scalar.dma_start(dw_d.ap(), dwout[:])


def kernel(W, T, data, labels):
    W = np.asarray(W, np.float32)
    T = np.asarray(T, np.float32)
    data = np.asarray(data, np.float32)
    labels = np.asarray(labels, np.int32)

    import ml_dtypes
    bf16 = ml_dtypes.bfloat16

    ET = np.exp(T).astype(np.float32)
    ETs = (ET / CHAT).astype(np.float32)
    etf = np.zeros((128, 128), np.float32)
    etb = np.zeros((128, 128), np.float32)
    for c in range(4):
        etf[32 * c:32 * c + 32, 32 * c:32 * c + 32] = ETs
        etb[32 * c:32 * c + 32, 32 * c:32 * c + 32] = ETs.T
    oz = np.zeros((128, 4), np.float32)
    ob = np.zeros((4, 128), np.float32)
    on32 = np.zeros((128, K), np.float32)
    for c in range(4):
        oz[32 * c:32 * c + 32, c] = 1.0
        ob[c, 32 * c:32 * c + 32] = 1.0
        on32[32 * c:32 * c + 32, :] = np.eye(K, dtype=np.float32)
    id128 = np.eye(128, dtype=np.float32)
    wt4 = np.zeros((128, 4, 128), np.float32)
    for g in range(4):
        for c in range(4):
            wt4[:, g, 32 * c:32 * c + 32] = W.T[128 * g:128 * g + 128, :]

    nc = _CACHE.get("nc")
    if nc is None:
        nc = _build_module()
        _CACHE["nc"] = nc

    in_maps = []
    for core in range(NC):
        dcore = data[core * WPC:(core + 1) * WPC]        # [256, 64, 512]
        lcore = labels[core * WPC:(core + 1) * WPC]
        dc = dcore.reshape(4, 64, 64, D)                 # [c, wg, i, d]
        # dt4[q, p, (4c+g)*512 + f'] = data[c, wg, 8q+i', 128g+p]
        #   where f' = 64*i' + wg, i = 8q + i'
        # dc -> [c, g, p, q, i', wg]
        dtt = dc.transpose(0, 3, 2, 1).reshape(4, 4, 128, 8, 8, 64)
        # dims now [c, g, p, q, i', wg] -> want [q, p, c, g, i', wg]
        dt4 = np.ascontiguousarray(dtt.transpose(3, 2, 0, 1, 4, 5)
                                   ).reshape(8, 128, 8192)
        # dn2[p, j*512 + d] = data row (128j + p) in position-major order
        dnn = dc.transpose(0, 2, 1, 3).reshape(RPC, D)   # [4096c+64i+wg, d]
        dn2 = np.ascontiguousarray(
            dnn.reshape(128, 128, D).transpose(1, 0, 2)).reshape(128, 65536)
        lc = lcore.reshape(4, 64, 64).transpose(0, 2, 1)  # [c, i, wg]
        oh = np.zeros((128, 4096), np.float32)
        ci, ii, wi = np.meshgrid(np.arange(4), np.arange(64), np.arange(64),
                                 indexing="ij")
        oh[32 * ci.ravel() + lc.ravel(), (64 * ii + wi).ravel()] = 1.0
        in_maps.append({
            "dt": dt4.astype(bf16), "dn": dn2.astype(bf16),
            "wt4": wt4.astype(bf16),
            "etf": etf.astype(bf16), "etb": etb.astype(bf16),
            "oz": oz.astype(bf16), "ob": ob, "on32": on32,
            "id128": id128.astype(bf16), "oh": oh.astype(bf16),
        })

    _CACHE["last_in_maps"] = in_maps
    res = run_bass_kernel_spmd(nc, in_maps, list(range(NC)))
    results = res.results

    dw_sum = np.zeros((K, D), np.float64)
    Mmat = np.zeros((K, K), np.float64)
    for core in range(NC):
        r = results[core]
        dw_sum += r["dw"].astype(np.float64)
        ae = r["ae"].astype(np.float64)   # [128, 4096] packed bf16
        be = r["be"].astype(np.float64)
        z = ae[:, 4032:4096].reshape(4, K, 64).sum(axis=1)   # [4, 64]
        rz = 1.0 / z
        ae_n = ae.reshape(4, K, 64, 64).transpose(0, 2, 3, 1)  # [c,i,wg,k]
        be_n = be.reshape(4, K, 64, 64).transpose(0, 2, 3, 1)
        Mmat += np.einsum("ciwk,ciwj,cw->kj",
                          ae_n[:, :M - 1], be_n[:, 1:], rz)

    counts = np.zeros((K, K), np.float64)
    np.add.at(counts, (labels[:, :-1].ravel(), labels[:, 1:].ravel()), 1.0)

    meandw = (-dw_sum / N).astype(np.float32)
    meandT = ((counts - (ET.astype(np.float64) / CHAT) * Mmat) / N
              ).astype(np.float32)
    return np.concatenate([meandw.ravel(), meandT.ravel()]).astype(np.float32)


# revision 17
# speedup vs baseline: 1.0082x; 1.0082x over previous
"""CRF layer gradient kernel for 8 TRN2 NeuronCores (v3).

Data-parallel over N=2048 words (256/core = 4 chains x 64 words).
Scaled forward-backward in exp domain, ETs = exp(T)/c:
  AE[i+1] = (AE[i] @ ETs) * E[i+1]   (AE[0] = E[0])
  BE[i-1] = (ETs @ BE[i]) * E[i-1]   (BE[63] = E[63], unnormalized)
fw/bw scans run CONCURRENTLY (z = sum_k AE[63] folded in afterwards via
ez = einv * (1/z) broadcast). Packing [128 = 4c x 32k, 4096 = 64 pos x 64 wg]
(POSITION-major: scan slices contiguous).

v3 perf structure:
- dt loaded as 8 x 2MB q-major units (host layout [8,128,8192]) split
  across both HWDGE rings (sync: q0-3, scalar: q7-4) -> big DMAs at line rate.
- dn loaded as 16 x 1MB partition-major units on the sync ring after dt.
- scan EMITTED INTERLEAVED with dots units so the PE FIFO never blocks the
  scan behind not-yet-needed dots matmuls; scan trails dt arrival.
- scan state in bf16 (fast matmuls), dots/dw matmuls col-tiled bf16.
- phase E per 128-column block: p1 -> G -> PE transpose -> col-tiled dw
  accumulation; dw reduced on device; ae/be out in bf16 on gpsimd ring.
"""

import sys

import numpy as np

sys.path.insert(0, "/opt/trn_rl_repo")

import concourse.bass as bass
import concourse.tile as tile
from concourse import bacc, mybir
from concourse.bass_utils import run_bass_kernel_spmd

N, M, K, D = 2048, 64, 32, 512
NC = 8
WPC = N // NC          # 256 words per core
RPC = WPC * M          # 16384 rows per core
CHAT = 60.0
F32 = mybir.dt.float32
BF16 = mybir.dt.bfloat16

_CACHE = {}


def _build_module():
    nc = bacc.Bacc("TRN2", target_bir_lowering=False, debug=False)

    dt_d = nc.dram_tensor("dt", [8, 128, 8192], BF16, kind="ExternalInput")
    dn_d = nc.dram_tensor("dn", [128, 65536], BF16, kind="ExternalInput")
    wt4_d = nc.dram_tensor("wt4", [128, 4, 128], BF16, kind="ExternalInput")
    etf_d = nc.dram_tensor("etf", [128, 128], BF16, kind="ExternalInput")
    etb_d = nc.dram_tensor("etb", [128, 128], BF16, kind="ExternalInput")
    oz_d = nc.dram_tensor("oz", [128, 4], BF16, kind="ExternalInput")
    ob_d = nc.dram_tensor("ob", [4, 128], F32, kind="ExternalInput")
    on32_d = nc.dram_tensor("on32", [128, K], F32, kind="ExternalInput")
    id128_d = nc.dram_tensor("id128", [128, 128], BF16, kind="ExternalInput")
    oh_d = nc.dram_tensor("oh", [128, 4096], BF16, kind="ExternalInput")
    dw_d = nc.dram_tensor("dw", [K, D], F32, kind="ExternalOutput")
    ae_d = nc.dram_tensor("ae", [128, 4096], BF16, kind="ExternalOutput")
    be_d = nc.dram_tensor("be", [128, 4096], BF16, kind="ExternalOutput")

    with tile.TileContext(nc) as tc:
        _kernel_body(tc, nc, dt_d, dn_d, wt4_d, etf_d, etb_d, oz_d, ob_d,
                     on32_d, id128_d, oh_d, dw_d, ae_d, be_d)
    nc.compile()
    return nc


def _kernel_body(tc, nc, dt_d, dn_d, wt4_d, etf_d, etb_d, oz_d, ob_d,
                 on32_d, id128_d, oh_d, dw_d, ae_d, be_d):
    from contextlib import ExitStack
    ctx = ExitStack()
    with ctx:
        consts = ctx.enter_context(tc.tile_pool(name="consts", bufs=1))
        big = ctx.enter_context(tc.tile_pool(name="big", bufs=1))
        dtp = ctx.enter_context(tc.tile_pool(name="dtp", bufs=4))
        dnp = ctx.enter_context(tc.tile_pool(name="dnp", bufs=10))
        scr = ctx.enter_context(tc.tile_pool(name="scr", bufs=6))
        gsbp = ctx.enter_context(tc.tile_pool(name="gsbp", bufs=3))

        # consts on the gpsimd (SWDGE) queue, wt4/etf/etb first (needed soon)
        wt4_t = consts.tile([128, 4, 128], BF16)
        nc.gpsimd.dma_start(wt4_t[:], wt4_d.ap())
        etf_t = consts.tile([128, 128], BF16)
        nc.gpsimd.dma_start(etf_t[:], etf_d.ap())
        etb_t = consts.tile([128, 128], BF16)
        nc.gpsimd.dma_start(etb_t[:], etb_d.ap())
        oz_t = consts.tile([128, 4], BF16)
        nc.gpsimd.dma_start(oz_t[:], oz_d.ap())
        ob_t = consts.tile([4, 128], F32)
        nc.gpsimd.dma_start(ob_t[:], ob_d.ap())
        on32_t = consts.tile([128, K], F32)
        nc.gpsimd.dma_start(on32_t[:], on32_d.ap())
        id128_t = consts.tile([128, 128], BF16)
        nc.gpsimd.dma_start(id128_t[:], id128_d.ap())
        oh_t = big.tile([128, 4096], BF16, tag="oh")

        e_t = big.tile([128, 4096], BF16, tag="e")
        ez_t = big.tile([128, 4096], BF16, tag="ez")   # einv, then einv*rz
        ae_t = big.tile([128, 4096], BF16, tag="ae")
        be_t = big.tile([128, 4096], BF16, tag="be")
        rzb_t = consts.tile([128, 64], F32)
        rz_t = consts.tile([4, 64], F32)

        # ---- input DMAs: dt on both rings (tiles created in need-order
        # so pool bufs go to the earliest-needed units), dn split on both ----
        dt_tiles = {}
        for q in (0, 7, 1, 6, 2, 5, 3, 4):
            dt_tiles[q] = dtp.tile([128, 8192], BF16, tag="dt", name=f"dt{q}")
            eng = nc.sync if q in (0, 1, 2, 3) else nc.scalar
            eng.dma_start(dt_tiles[q][:], dt_d.ap()[q])
        # oh only needed in phase E; load it on the scalar ring after dt
        nc.scalar.dma_start(oh_t[:], oh_d.ap())
        dn_tiles = {}
        for u in (0, 4, 8, 12, 2, 6, 10, 14, 1, 5, 9, 13, 3, 7, 11, 15):
            dn_tiles[u] = dnp.tile([128, 8, 512], BF16, tag="dn",
                                   name=f"dn{u}")
            eng = nc.sync if u in (0, 4, 8, 12, 1, 5, 9, 13) else nc.scalar
            eng.dma_start(
                dn_tiles[u][:],
                dn_d.ap()[:, 4096 * u:4096 * u + 4096]
                .rearrange("p (j d) -> p j d", j=8))

        dot_psum = {}

        def emit_unit(dotp, q):
            """dots for positions 8q..8q+8: 16 col-tiled MMs + exp ACTs."""
            P = dotp.tile([128, 512], F32)
            dot_psum[q] = P
            for g in range(4):
                for c in range(4):
                    nc.tensor.matmul(
                        P[32 * c:32 * c + 32, :],
                        wt4_t[:, g, 32 * c:32 * c + 32],
                        dt_tiles[q][:, 512 * (4 * c + g):512 * (4 * c + g) + 512],
                        start=(g == 0), stop=(g == 3),
                        tile_position=(0, 32 * c))
            sl = slice(512 * q, 512 * q + 512)
            nc.scalar.activation(e_t[:, sl], P[:],
                                 mybir.ActivationFunctionType.Exp)
            nc.scalar.activation(ez_t[:, sl], P[:],
                                 mybir.ActivationFunctionType.Exp,
                                 scale=-1.0)

        # ---- Phase A+B interleaved: dots units woven into the scan ----
        with tc.tile_pool(name="dotp", bufs=3, space="PSUM") as dotp, \
             tc.tile_pool(name="scanp", bufs=3, space="PSUM") as scanp, \
             tc.tile_pool(name="zp", bufs=1, space="PSUM") as zp:
            emit_unit(dotp, 0)
            emit_unit(dotp, 7)

            nc.vector.tensor_copy(ae_t[:, 0:64], e_t[:, 0:64])
            nc.vector.tensor_copy(be_t[:, 4032:4096], e_t[:, 4032:4096])
            af = scanp.tile([128, 64], F32, tag="s")
            nc.tensor.matmul(af[:], etf_t[:], ae_t[:, 0:64],
                             start=True, stop=True)
            bb = scanp.tile([128, 64], F32, tag="s")
            nc.tensor.matmul(bb[:], etb_t[:], be_t[:, 4032:4096],
                             start=True, stop=True)

            unit_sched = {8: (1, 6), 16: (2, 5), 24: (3, 4)}
            for s in range(1, 64):
                if s in unit_sched:
                    for q in unit_sched[s]:
                        emit_unit(dotp, q)
                sf = slice(64 * s, 64 * s + 64)
                sb = slice(64 * (63 - s), 64 * (63 - s) + 64)
                nc.vector.tensor_mul(ae_t[:, sf], af[:], e_t[:, sf])
                nc.vector.tensor_mul(be_t[:, sb], bb[:], e_t[:, sb])
                if s < 63:
                    af = scanp.tile([128, 64], F32, tag="s")
                    nc.tensor.matmul(af[:], etf_t[:], ae_t[:, sf],
                                     start=True, stop=True)
                    bb = scanp.tile([128, 64], F32, tag="s")
                    nc.tensor.matmul(bb[:], etb_t[:], be_t[:, sb],
                                     start=True, stop=True)

            nc.scalar.dma_start(ae_d.ap(), ae_t[:])
            nc.scalar.dma_start(be_d.ap(), be_t[:])

            # ---- z and ez = einv * (1/z) broadcast ----
            z_ps = zp.tile([128, 64], F32, tag="z")
            nc.tensor.matmul(z_ps[0:4, :], oz_t[:], ae_t[:, 4032:4096],
                             start=True, stop=True)
            nc.vector.reciprocal(rz_t[:], z_ps[0:4, :])
            rzb_ps = zp.tile([128, 64], F32, tag="z")
            nc.tensor.matmul(rzb_ps[:], ob_t[:], rz_t[:],
                             start=True, stop=True)
            nc.vector.tensor_copy(rzb_t[:], rzb_ps[:])

        ez3 = ez_t[:].rearrange("p (i w) -> p i w", i=64)
        rz3 = rzb_t[:].unsqueeze(1)
        rz3b, ez3b = bass.broadcast_tensor_aps(rz3, ez3)
        nc.vector.tensor_mul(ez3, ez3b, rz3b)        # in-place einv -> ez

        # ---- Phase E: G = p1 - oh; PE transpose; col-tiled dw ----
        with tc.tile_pool(name="trp", bufs=2, space="PSUM") as trp, \
             tc.tile_pool(name="dwp", bufs=1, space="PSUM") as dwp, \
             tc.tile_pool(name="drp", bufs=1, space="PSUM") as drp:
            dwacc = dwp.tile([128, 512], F32)
            for jj in range(32):
                sl = slice(128 * jj, 128 * jj + 128)
                p1c = scr.tile([128, 128], F32, tag="p1")
                nc.vector.tensor_mul(p1c[:], ae_t[:, sl], be_t[:, sl])
                p1b = scr.tile([128, 128], BF16, tag="p1b")
                nc.vector.tensor_mul(p1b[:], p1c[:], ez_t[:, sl])
                gc = scr.tile([128, 128], BF16, tag="g")
                nc.vector.tensor_sub(gc[:], p1b[:], oh_t[:, sl])
                tr = trp.tile([128, 128], BF16)
                nc.tensor.transpose(tr[:], gc[:], id128_t[:])
                gsb = gsbp.tile([128, 128], BF16)
                nc.scalar.activation(gsb[:], tr[:],
                                     mybir.ActivationFunctionType.Copy)
                for c in range(4):
                    j = 32 * c + jj
                    u, slot = j // 8, j % 8
                    nc.tensor.matmul(dwacc[32 * c:32 * c + 32, :],
                                     gsb[:, 32 * c:32 * c + 32],
                                     dn_tiles[u][:, slot, :],
                                     start=(jj == 0), stop=(jj == 31),
                                     tile_position=(0, 32 * c))

            dwsb = gsbp.tile([128, 512], F32, tag="dwsb")
            nc.vector.tensor_copy(dwsb[:], dwacc[:])
            dwred = drp.tile([K, 512], F32)
            nc.tensor.matmul(dwred[:], on32_t[:], dwsb[:],
                             start=True, stop=True)
            dwout = gsbp.tile([K, 512], F32, tag="dwout")
            nc.vector.tensor_copy(dwout[:], dwred[:])
            nc.scalar.dma_start(dw_d.ap(), dwout[:])


def kernel(W, T, data, labels):
    W = np.asarray(W, np.float32)
    T = np.asarray(T, np.float32)
    data = np.asarray(data, np.float32)
    labels = np.asarray(labels, np.int32)

    import ml_dtypes
    bf16 = ml_dtypes.bfloat16

    ET = np.exp(T).astype(np.float32)
    ETs = (ET / CHAT).astype(np.float32)
    etf = np.zeros((128, 128), np.float32)
    etb = np.zeros((128, 128), np.float32)
    for c in range(4):
        etf[32 * c:32 * c + 32, 32 * c:32 * c + 32] = ETs
        etb[32 * c:32 * c + 32, 32 * c:32 * c + 32] = ETs.T
    oz = np.zeros((128, 4), np.float32)
    ob = np.zeros((4, 128), np.float32)
    on32 = np.zeros((128, K), np.float32)
    for c in range(4):
        oz[32 * c:32 * c + 32, c] = 1.0
        ob[c, 32 * c:32 * c + 32] = 1.0
        on32[32 * c:32 * c + 32, :] = np.eye(K, dtype=np.float32)
    id128 = np.eye(128, dtype=np.float32)
    wt4 = np.zeros((128, 4, 128), np.float32)
    for g in range(4):
        for c in range(4):
            wt4[:, g, 32 * c:32 * c + 32] = W.T[128 * g:128 * g + 128, :]

    nc = _CACHE.get("nc")
    if nc is None:
        nc = _build_module()
        _CACHE["nc"] = nc

    in_maps = []
    for core in range(NC):
        dcore = data[core * WPC:(core + 1) * WPC]        # [256, 64, 512]
        lcore = labels[core * WPC:(core + 1) * WPC]
        dc = dcore.reshape(4, 64, 64, D)                 # [c, wg, i, d]
        # dt4[q, p, (4c+g)*512 + f'] = data[c, wg, 8q+i', 128g+p]
        #   where f' = 64*i' + wg, i = 8q + i'
        # dc -> [c, g, p, q, i', wg]
        dtt = dc.transpose(0, 3, 2, 1).reshape(4, 4, 128, 8, 8, 64)
        # dims now [c, g, p, q, i', wg] -> want [q, p, c, g, i', wg]
        dt4 = np.ascontiguousarray(dtt.transpose(3, 2, 0, 1, 4, 5)
                                   ).reshape(8, 128, 8192)
        # dn2[p, j*512 + d] = data row (128j + p) in position-major order
        dnn = dc.transpose(0, 2, 1, 3).reshape(RPC, D)   # [4096c+64i+wg, d]
        dn2 = np.ascontiguousarray(
            dnn.reshape(128, 128, D).transpose(1, 0, 2)).reshape(128, 65536)
        lc = lcore.reshape(4, 64, 64).transpose(0, 2, 1)  # [c, i, wg]
        oh = np.zeros((128, 4096), np.float32)
        ci, ii, wi = np.meshgrid(np.arange(4), np.arange(64), np.arange(64),
                                 indexing="ij")
        oh[32 * ci.ravel() + lc.ravel(), (64 * ii + wi).ravel()] = 1.0
        in_maps.append({
            "dt": dt4.astype(bf16), "dn": dn2.astype(bf16),
            "wt4": wt4.astype(bf16),
            "etf": etf.astype(bf16), "etb": etb.astype(bf16),
            "oz": oz.astype(bf16), "ob": ob, "on32": on32,
            "id128": id128.astype(bf16), "oh": oh.astype(bf16),
        })

    _CACHE["last_in_maps"] = in_maps
    res = run_bass_kernel_spmd(nc, in_maps, list(range(NC)))
    results = res.results

    dw_sum = np.zeros((K, D), np.float64)
    Mmat = np.zeros((K, K), np.float64)
    for core in range(NC):
        r = results[core]
        dw_sum += r["dw"].astype(np.float64)
        ae = r["ae"].astype(np.float64)   # [128, 4096] packed bf16
        be = r["be"].astype(np.float64)
        z = ae[:, 4032:4096].reshape(4, K, 64).sum(axis=1)   # [4, 64]
        rz = 1.0 / z
        ae_n = ae.reshape(4, K, 64, 64).transpose(0, 2, 3, 1)  # [c,i,wg,k]
        be_n = be.reshape(4, K, 64, 64).transpose(0, 2, 3, 1)
        Mmat += np.einsum("ciwk,ciwj,cw->kj",
                          ae_n[:, :M - 1], be_n[:, 1:], rz)

    counts = np.zeros((K, K), np.float64)
    np.add.at(counts, (labels[:, :-1].ravel(), labels[:, 1:].ravel()), 1.0)

    meandw = (-dw_sum / N).astype(np.float32)
    meandT = ((counts - (ET.astype(np.float64) / CHAT) * Mmat) / N
              ).astype(np.float32)
    return np.concatenate([meandw.ravel(), meandT.ravel()]).astype(np.float32)


# revision 26
# speedup vs baseline: 1.1685x; 1.1590x over previous
"""CRF layer gradient kernel for 8 TRN2 NeuronCores (v3).

Data-parallel over N=2048 words (256/core = 4 chains x 64 words).
Scaled forward-backward in exp domain, ETs = exp(T)/c:
  AE[i+1] = (AE[i] @ ETs) * E[i+1]   (AE[0] = E[0])
  BE[i-1] = (ETs @ BE[i]) * E[i-1]   (BE[63] = E[63], unnormalized)
fw/bw scans run CONCURRENTLY (z = sum_k AE[63] folded in afterwards via
ez = einv * (1/z) broadcast). Packing [128 = 4c x 32k, 4096 = 64 pos x 64 wg]
(POSITION-major: scan slices contiguous).

v3 perf structure:
- dt loaded as 8 x 2MB q-major units (host layout [8,128,8192]) split
  across both HWDGE rings (sync: q0-3, scalar: q7-4) -> big DMAs at line rate.
- dn loaded as 16 x 1MB partition-major units on the sync ring after dt.
- scan EMITTED INTERLEAVED with dots units so the PE FIFO never blocks the
  scan behind not-yet-needed dots matmuls; scan trails dt arrival.
- scan state in bf16 (fast matmuls), dots/dw matmuls col-tiled bf16.
- phase E per 128-column block: p1 -> G -> PE transpose -> col-tiled dw
  accumulation; dw reduced on device; ae/be out in bf16 on gpsimd ring.
"""

import sys

import numpy as np

sys.path.insert(0, "/opt/trn_rl_repo")

import concourse.bass as bass
import concourse.tile as tile
from concourse import bacc, mybir
from concourse.bass_utils import run_bass_kernel_spmd

N, M, K, D = 2048, 64, 32, 512
NC = 8
WPC = N // NC          # 256 words per core
RPC = WPC * M          # 16384 rows per core
CHAT = 60.0
F32 = mybir.dt.float32
BF16 = mybir.dt.bfloat16

_CACHE = {}


def _build_module():
    nc = bacc.Bacc("TRN2", target_bir_lowering=False, debug=False)

    dt_d = nc.dram_tensor("dt", [8, 128, 8192], BF16, kind="ExternalInput")
    dn_d = nc.dram_tensor("dn", [16, 128, 4096], BF16, kind="ExternalInput")
    wt4_d = nc.dram_tensor("wt4", [128, 4, 128], BF16, kind="ExternalInput")
    etf_d = nc.dram_tensor("etf", [128, 128], BF16, kind="ExternalInput")
    etb_d = nc.dram_tensor("etb", [128, 128], BF16, kind="ExternalInput")
    oz_d = nc.dram_tensor("oz", [128, 4], BF16, kind="ExternalInput")
    ob_d = nc.dram_tensor("ob", [4, 128], F32, kind="ExternalInput")
    on32_d = nc.dram_tensor("on32", [128, K], F32, kind="ExternalInput")
    id128_d = nc.dram_tensor("id128", [128, 128], BF16, kind="ExternalInput")
    oh_d = nc.dram_tensor("oh", [128, 4096], BF16, kind="ExternalInput")
    dw_d = nc.dram_tensor("dw", [K, D], F32, kind="ExternalOutput")
    ae_d = nc.dram_tensor("ae", [128, 4096], BF16, kind="ExternalOutput")
    be_d = nc.dram_tensor("be", [128, 4096], BF16, kind="ExternalOutput")

    with tile.TileContext(nc) as tc:
        _kernel_body(tc, nc, dt_d, dn_d, wt4_d, etf_d, etb_d, oz_d, ob_d,
                     on32_d, id128_d, oh_d, dw_d, ae_d, be_d)
    nc.compile()
    return nc


def _kernel_body(tc, nc, dt_d, dn_d, wt4_d, etf_d, etb_d, oz_d, ob_d,
                 on32_d, id128_d, oh_d, dw_d, ae_d, be_d):
    from contextlib import ExitStack
    ctx = ExitStack()
    with ctx:
        consts = ctx.enter_context(tc.tile_pool(name="consts", bufs=1))
        big = ctx.enter_context(tc.tile_pool(name="big", bufs=1))
        dtp = ctx.enter_context(tc.tile_pool(name="dtp", bufs=5))
        dnp = ctx.enter_context(tc.tile_pool(name="dnp", bufs=8))
        scr = ctx.enter_context(tc.tile_pool(name="scr", bufs=6))
        gsbp = ctx.enter_context(tc.tile_pool(name="gsbp", bufs=3))

        # consts on the gpsimd (SWDGE) queue, wt4/etf/etb first (needed soon)
        wt4_t = consts.tile([128, 4, 128], BF16)
        nc.gpsimd.dma_start(wt4_t[:], wt4_d.ap())
        etf_t = consts.tile([128, 128], BF16)
        nc.gpsimd.dma_start(etf_t[:], etf_d.ap())
        etb_t = consts.tile([128, 128], BF16)
        nc.gpsimd.dma_start(etb_t[:], etb_d.ap())
        oz_t = consts.tile([128, 4], BF16)
        nc.gpsimd.dma_start(oz_t[:], oz_d.ap())
        ob_t = consts.tile([4, 128], F32)
        nc.gpsimd.dma_start(ob_t[:], ob_d.ap())
        on32_t = consts.tile([128, K], F32)
        nc.gpsimd.dma_start(on32_t[:], on32_d.ap())
        id128_t = consts.tile([128, 128], BF16)
        nc.gpsimd.dma_start(id128_t[:], id128_d.ap())
        oh_t = big.tile([128, 4096], BF16, tag="oh")

        e_t = big.tile([128, 4096], BF16, tag="e")     # E; then ae*be scratch
        ez_t = big.tile([128, 4096], BF16, tag="ez")   # einv, then einv*rz
        ae_t = big.tile([128, 4096], BF16, tag="ae")
        be_t = big.tile([128, 4096], BF16, tag="be")
        rzb_t = consts.tile([128, 64], BF16)
        rz_t = consts.tile([4, 64], F32)

        # ---- input DMAs. Stream separation is critical: a pool-blocked
        # dma issue stalls its whole engine queue, so the scalar (ACT)
        # stream must stay free for exp ACTs / phase-E copies.
        #   sync ring:   dt q0-3, then all 16 dn units
        #   gpsimd ring: consts (above), dt q7-4
        #   scalar ring: oh only (then ACTs, later outputs)
        dt_tiles = {}
        for q in (0, 7, 1, 6, 2, 5, 3, 4):
            dt_tiles[q] = dtp.tile([128, 8192], BF16, tag="dt", name=f"dt{q}")
            eng = nc.sync if q in (0, 1, 2, 3) else nc.gpsimd
            eng.dma_start(dt_tiles[q][:], dt_d.ap()[q])
        nc.scalar.dma_start(oh_t[:], oh_d.ap())
        dn_tiles = {}
        for u in range(16):
            dn_tiles[u] = dnp.tile([128, 8, 512], BF16, tag="dn",
                                   name=f"dn{u}")
            nc.sync.dma_start(
                dn_tiles[u][:],
                dn_d.ap()[u].rearrange("p (s d) -> p s d", s=8))

        dot_psum = {}

        def emit_unit(dotp, q):
            """dots for positions 8q..8q+8: 16 col-tiled MMs + exp ACTs."""
            P = dotp.tile([128, 512], F32)
            dot_psum[q] = P
            for g in range(4):
                for c in range(4):
                    nc.tensor.matmul(
                        P[32 * c:32 * c + 32, :],
                        wt4_t[:, g, 32 * c:32 * c + 32],
                        dt_tiles[q][:, 512 * (4 * c + g):512 * (4 * c + g) + 512],
                        start=(g == 0), stop=(g == 3),
                        tile_position=(0, 32 * c))
            sl = slice(512 * q, 512 * q + 512)
            nc.scalar.activation(e_t[:, sl], P[:],
                                 mybir.ActivationFunctionType.Exp)
            nc.scalar.activation(ez_t[:, sl], P[:],
                                 mybir.ActivationFunctionType.Exp,
                                 scale=-1.0)

        # ---- Phase A+B interleaved: dots units woven into the scan ----
        with tc.tile_pool(name="dotp", bufs=3, space="PSUM") as dotp, \
             tc.tile_pool(name="scanp", bufs=3, space="PSUM") as scanp, \
             tc.tile_pool(name="zp", bufs=1, space="PSUM") as zp:
            emit_unit(dotp, 0)
            emit_unit(dotp, 7)

            nc.vector.tensor_copy(ae_t[:, 0:64], e_t[:, 0:64])
            nc.vector.tensor_copy(be_t[:, 4032:4096], e_t[:, 4032:4096])
            af = scanp.tile([128, 64], F32, tag="s")
            nc.tensor.matmul(af[:], etf_t[:], ae_t[:, 0:64],
                             start=True, stop=True)
            bb = scanp.tile([128, 64], F32, tag="s")
            nc.tensor.matmul(bb[:], etb_t[:], be_t[:, 4032:4096],
                             start=True, stop=True)

            unit_sched = {8: (1, 6), 16: (2, 5), 24: (3, 4)}
            for s in range(1, 64):
                if s in unit_sched:
                    for q in unit_sched[s]:
                        emit_unit(dotp, q)
                sf = slice(64 * s, 64 * s + 64)
                sb = slice(64 * (63 - s), 64 * (63 - s) + 64)
                nc.vector.tensor_mul(ae_t[:, sf], af[:], e_t[:, sf])
                nc.vector.tensor_mul(be_t[:, sb], bb[:], e_t[:, sb])
                if s < 63:
                    af = scanp.tile([128, 64], F32, tag="s")
                    nc.tensor.matmul(af[:], etf_t[:], ae_t[:, sf],
                                     start=True, stop=True)
                    bb = scanp.tile([128, 64], F32, tag="s")
                    nc.tensor.matmul(bb[:], etb_t[:], be_t[:, sb],
                                     start=True, stop=True)

            # e_t is dead after the scan: reuse it for ae*be (bf16, DVE)
            # while the PE does the z matmuls
            nc.vector.tensor_mul(e_t[:], ae_t[:], be_t[:])
            nc.scalar.dma_start(ae_d.ap(), ae_t[:])
            nc.scalar.dma_start(be_d.ap(), be_t[:])

            # ---- z and ez = einv * (1/z) broadcast ----
            z_ps = zp.tile([128, 64], F32, tag="z")
            nc.tensor.matmul(z_ps[0:4, :], oz_t[:], ae_t[:, 4032:4096],
                             start=True, stop=True)
            nc.vector.reciprocal(rz_t[:], z_ps[0:4, :])
            rzb_ps = zp.tile([128, 64], F32, tag="z")
            nc.tensor.matmul(rzb_ps[:], ob_t[:], rz_t[:],
                             start=True, stop=True)
            nc.vector.tensor_copy(rzb_t[:], rzb_ps[:])

        ez3 = ez_t[:].rearrange("p (i w) -> p i w", i=64)
        rz3 = rzb_t[:].unsqueeze(1)
        rz3b, ez3b = bass.broadcast_tensor_aps(rz3, ez3)
        nc.vector.tensor_mul(ez3, ez3b, rz3b)        # in-place einv -> ez

        # ---- Phase E: G = p1 - oh; PE transpose; col-tiled dw ----
        with tc.tile_pool(name="trp", bufs=2, space="PSUM") as trp, \
             tc.tile_pool(name="dwp", bufs=1, space="PSUM") as dwp, \
             tc.tile_pool(name="drp", bufs=1, space="PSUM") as drp:
            dwacc = dwp.tile([128, 512], F32)
            for jj in range(32):
                sl = slice(128 * jj, 128 * jj + 128)
                p1b = scr.tile([128, 128], BF16, tag="p1b")
                nc.vector.tensor_mul(p1b[:], e_t[:, sl], ez_t[:, sl])
                gc = scr.tile([128, 128], BF16, tag="g")
                nc.vector.tensor_sub(gc[:], p1b[:], oh_t[:, sl])
                tr = trp.tile([128, 128], BF16)
                nc.tensor.transpose(tr[:], gc[:], id128_t[:])
                gsb = gsbp.tile([128, 128], BF16)
                nc.scalar.activation(gsb[:], tr[:],
                                     mybir.ActivationFunctionType.Copy)
                for c in range(4):
                    u, slot = jj // 2, 4 * (jj % 2) + c
                    nc.tensor.matmul(dwacc[32 * c:32 * c + 32, :],
                                     gsb[:, 32 * c:32 * c + 32],
                                     dn_tiles[u][:, slot, :],
                                     start=(jj == 0), stop=(jj == 31),
                                     tile_position=(0, 32 * c))

            dwsb = gsbp.tile([128, 512], F32, tag="dwsb")
            nc.vector.tensor_copy(dwsb[:], dwacc[:])
            dwred = drp.tile([K, 512], F32)
            nc.tensor.matmul(dwred[:], on32_t[:], dwsb[:],
                             start=True, stop=True)
            dwout = gsbp.tile([K, 512], F32, tag="dwout")
            nc.vector.tensor_copy(dwout[:], dwred[:])
            nc.scalar.dma_start(dw_d.ap(), dwout[:])


def kernel(W, T, data, labels):
    W = np.asarray(W, np.float32)
    T = np.asarray(T, np.float32)
    data = np.asarray(data, np.float32)
    labels = np.asarray(labels, np.int32)

    import ml_dtypes
    bf16 = ml_dtypes.bfloat16

    ET = np.exp(T).astype(np.float32)
    ETs = (ET / CHAT).astype(np.float32)
    etf = np.zeros((128, 128), np.float32)
    etb = np.zeros((128, 128), np.float32)
    for c in range(4):
        etf[32 * c:32 * c + 32, 32 * c:32 * c + 32] = ETs
        etb[32 * c:32 * c + 32, 32 * c:32 * c + 32] = ETs.T
    oz = np.zeros((128, 4), np.float32)
    ob = np.zeros((4, 128), np.float32)
    on32 = np.zeros((128, K), np.float32)
    for c in range(4):
        oz[32 * c:32 * c + 32, c] = 1.0
        ob[c, 32 * c:32 * c + 32] = 1.0
        on32[32 * c:32 * c + 32, :] = np.eye(K, dtype=np.float32)
    id128 = np.eye(128, dtype=np.float32)
    wt4 = np.zeros((128, 4, 128), np.float32)
    for g in range(4):
        for c in range(4):
            wt4[:, g, 32 * c:32 * c + 32] = W.T[128 * g:128 * g + 128, :]

    nc = _CACHE.get("nc")
    if nc is None:
        nc = _build_module()
        _CACHE["nc"] = nc

    in_maps = []
    for core in range(NC):
        dcore = data[core * WPC:(core + 1) * WPC]        # [256, 64, 512]
        lcore = labels[core * WPC:(core + 1) * WPC]
        dc = dcore.reshape(4, 64, 64, D)                 # [c, wg, i, d]
        # dt4[q, p, (4c+g)*512 + f'] = data[c, wg, 8q+i', 128g+p]
        #   where f' = 64*i' + wg, i = 8q + i'
        # dc -> [c, g, p, q, i', wg]
        dtt = dc.transpose(0, 3, 2, 1).reshape(4, 4, 128, 8, 8, 64)
        # dims now [c, g, p, q, i', wg] -> want [q, p, c, g, i', wg]
        dt4 = np.ascontiguousarray(dtt.transpose(3, 2, 0, 1, 4, 5)
                                   ).reshape(8, 128, 8192)
        # dn4[u, p, (4b+c)*512+d] = row (4096c + 128(2u+b) + p); unit=jj-pair
        dnn = dc.transpose(0, 2, 1, 3).reshape(RPC, D)   # [4096c+64i+wg, d]
        dn4 = np.ascontiguousarray(
            dnn.reshape(4, 16, 2, 128, D)                # [c, u, b, p, d]
            .transpose(1, 3, 2, 0, 4)).reshape(16, 128, 4096)
        lc = lcore.reshape(4, 64, 64).transpose(0, 2, 1)  # [c, i, wg]
        oh = np.zeros((128, 4096), np.float32)
        ci, ii, wi = np.meshgrid(np.arange(4), np.arange(64), np.arange(64),
                                 indexing="ij")
        oh[32 * ci.ravel() + lc.ravel(), (64 * ii + wi).ravel()] = 1.0
        in_maps.append({
            "dt": dt4.astype(bf16), "dn": dn4.astype(bf16),
            "wt4": wt4.astype(bf16),
            "etf": etf.astype(bf16), "etb": etb.astype(bf16),
            "oz": oz.astype(bf16), "ob": ob, "on32": on32,
            "id128": id128.astype(bf16), "oh": oh.astype(bf16),
        })

    _CACHE["last_in_maps"] = in_maps
    res = run_bass_kernel_spmd(nc, in_maps, list(range(NC)))
    results = res.results

    dw_sum = np.zeros((K, D), np.float64)
    Mmat = np.zeros((K, K), np.float64)
    for core in range(NC):
        r = results[core]
        dw_sum += r["dw"].astype(np.float64)
        ae = r["ae"].astype(np.float64)   # [128, 4096] packed bf16
        be = r["be"].astype(np.float64)
        z = ae[:, 4032:4096].reshape(4, K, 64).sum(axis=1)   # [4, 64]
        rz = 1.0 / z
        ae_n = ae.reshape(4, K, 64, 64).transpose(0, 2, 3, 1)  # [c,i,wg,k]
        be_n = be.reshape(4, K, 64, 64).transpose(0, 2, 3, 1)
        Mmat += np.einsum("ciwk,ciwj,cw->kj",
                          ae_n[:, :M - 1], be_n[:, 1:], rz)

    counts = np.zeros((K, K), np.float64)
    np.add.at(counts, (labels[:, :-1].ravel(), labels[:, 1:].ravel()), 1.0)

    meandw = (-dw_sum / N).astype(np.float32)
    meandT = ((counts - (ET.astype(np.float64) / CHAT) * Mmat) / N
              ).astype(np.float32)
    return np.concatenate([meandw.ravel(), meandT.ravel()]).astype(np.float32)
